# revision 70
# baseline (speedup 1.0000x reference)
"""Trainium2 Bass kernel for nn_CrossAttentionBlock.

Reference computation (B=16384, C=1024, D=128):
    g_x     = x0 @ g_w.T + g_b          # [B, D]
    theta_x = x1 @ theta_w.T + theta_b  # [B, D]
    phi_x   = x1 @ phi_w.T + phi_b      # [B, D]
    f[b,i,j] = phi_x[b,i] * theta_x[b,j]
    attn = softmax(f, axis=-1)
    y[b,i] = sum_j attn[b,i,j] * g_x[b,j]
    out = y @ W_w.T + W_b + x0          # [B, C]

Unnormalized form used on-chip (no max-subtraction needed: |f| <= ~40, exp
fits fp32/bf16 comfortably):
    E_T[j,i] = exp(theta[b,j] * phi[b,i])        (per b, j on partitions)
    num[i] = sum_j g[b,j] * E_T[j,i]   den[i] = sum_j E_T[j,i]
    y[b,i] = num[i] / den[i]

Sharding: pure data parallel over batch across 8 cores (2048 rows/core).

Per-core pipeline:
  P1: theta/phi projections -> tp16 [b-part, G, 256] fp16; g projection ->
      g_xT interleaved with ones into g1 [d, 2b] bf16.
  P2: outer products run TWO-concurrent via PE row tiling: each half of the
      array (tile_position (0,0) / (64,0), K=64) computes one 4-row quad's
      rank-1 outer per N=512 matmul. Quad rows sit at partitions 64s+16p
      (slot v along the free dim); the other 60 rows per half are zero so
      the zeros contribute nothing BUT keep the whole 128-row array active
      (the HAM activity monitor throttles the PE clock to 1.2 GHz when
      array activity is low -- small-K matmuls alone read as idle).
      f-tiles are 8 rows [128, 1024] fp32 = 2 PSUM banks, triple-buffered
      so the next outer never waits on the exp of two tiles ago.
      exp: of every 8 f-tiles, ACT (exact EXP) takes 5 and DVE (int16
      Schraudolph bit-trick) takes 3 -- tile-granular so the engines run
      concurrently. Per-b reduce matmuls (lhsT=E_T_b, rhs=[g|1]) accumulate
      num/den in PSUM; DVE fast-reciprocal+mul produce y_T [d,b] bf16.
  P3: final matmul (lhsT=y_T group, rhs=W_w.T, N=1024) + residual add + DMA.
"""

import os
from contextlib import ExitStack, nullcontext

import numpy as np

import concourse.bass as bass
import concourse.tile as tile
from concourse import bacc
from concourse import mybir

F32 = mybir.dt.float32
F16 = mybir.dt.float16
BF16 = mybir.dt.bfloat16
I16 = mybir.dt.int16

# bf16 Schraudolph exp: bf16_bits(e^f) ~ int16(f * 128*log2(e) + 16250.4).
# ~6% max relative error on weights; softmax ratio cancels most of it.
SCH_A = 128.0 * 1.4426950408889634
SCH_B = 16250.4

NCORES = 8
B, C, D = 16384, 1024, 128
KC = C // 128  # 8 contraction chunks for the projections

NSTRIP = 2          # concurrent K=64 half-array outer matmuls
FTILE = 4 * NSTRIP  # batch rows per f/E tile = 8
NSLOT = 16          # outer slots per realign batch (= one 128-row group)
# exp engine interleave: DVE takes these residues mod 8, ACT the rest
DVE_TILES = (1, 4, 6)
# free-dim width of the per-tile PE activity-filler matmul (0 disables)
FILL_N = int(os.environ.get("K_FILL", "256"))


def build_bass(bc: int, reps: int = 1):
    """Build the per-core bass program for a batch slice of `bc` rows."""
    ng = bc // 128  # groups of 128 rows
    qsz = min(bc, 512)
    nq = max(1, bc // qsz)
    n_ftiles = ng * NSLOT
    assert bc % 128 == 0

    nc = bacc.Bacc(trn_type="TRN2")

    # inputs are pre-swizzled on the host so every DMA lands per-partition
    # contiguous ([p, k*b] rows): x1g[G*128+p, k*128+b] = x1[G*128+b, k*128+p]
    x1g = nc.dram_tensor("x1g", [ng * 128, KC * 128], F16, kind="ExternalInput")
    x0g = nc.dram_tensor("x0g", [nq * 128, KC * qsz], F16, kind="ExternalInput")
    x0r = nc.dram_tensor("x0r", [bc, C], F16, kind="ExternalInput")
    wc = nc.dram_tensor("wc", [128, KC * 2 * D], F16, kind="ExternalInput")
    gwt = nc.dram_tensor("gwt", [128, KC * D], F16, kind="ExternalInput")
    wwt = nc.dram_tensor("wwt", [D, C], BF16, kind="ExternalInput")
    btp = nc.dram_tensor("btp", [128, 2 * D], F32, kind="ExternalInput")
    gb = nc.dram_tensor("gb", [D, 1], F32, kind="ExternalInput")
    out = nc.dram_tensor("out", [bc, C], F32, kind="ExternalOutput")

    with tile.TileContext(nc) as tc, ExitStack() as ctx:
        singles = ctx.enter_context(tc.tile_pool(name="singles", bufs=1))

        # static weights / consts (DMA emission deferred: queue order matters)
        wc_sb = singles.tile([128, KC, 2 * D], F16)  # [c-part, chunk, 256]
        btp_sb = singles.tile([128, 2 * D], F32)
        gwt_sb = singles.tile([128, KC, D], F16)
        wwt_sb = singles.tile([128, C], BF16)
        gb_sb = singles.tile([128, 1], F32)

        # persistent per-core activations
        tp16 = singles.tile([128, ng, 2 * D], F16)  # [theta|phi] fp16
        g1 = singles.tile([128, 2 * bc], BF16)  # g_xT interleaved with ones
        y16 = singles.tile([128, bc], BF16)  # y_T [d, b] bf16

        # ping-pong realign buffers (one 128-row group per batch).
        # thbuf[64s+16p, v*128+i] = theta[row 128G+64s+16p+v, i]; phbuf holds
        # phi block-diagonally at [64s+16p, v*512 + p*128 + c]; all other
        # partitions stay zero (zz-DMA once; realign rewrites only live rows).
        thbuf = [singles.tile([128, NSLOT * D], F16, name=f"thbuf{i}") for i in range(2)]
        phbuf = [
            singles.tile([128, NSLOT * 4 * D], F16, name=f"phbuf{i}") for i in range(2)
        ]
        # warm-up matmuls use a separate garbage-ok tile (no dependencies)
        wbuf = singles.tile([128, 5 * D], F16, name="warmbuf")
        nc.gpsimd.memset(wbuf, 0.0)

        rep_ctx = tc.For_i(0, reps, 1) if reps > 1 else nullcontext()
        with rep_ctx:
            with (
                tc.tile_pool(name="xin", bufs=6) as xin,
                tc.tile_pool(name="xg", bufs=2) as xg,
                tc.tile_pool(name="projpsum", bufs=1, space="PSUM") as projpsum,
                tc.tile_pool(name="fpsum", bufs=3, space="PSUM") as fpsum,
                tc.tile_pool(name="ndpsum", bufs=1, space="PSUM") as ndpsum,
                tc.tile_pool(name="epool", bufs=6) as epool,
                tc.tile_pool(name="ndsb", bufs=2) as ndsb,
                tc.tile_pool(name="rec", bufs=2) as rec,
                tc.tile_pool(name="resid", bufs=4) as resid,
                tc.tile_pool(name="osb", bufs=3) as osb,
            ):
                g1v = g1.rearrange("p (b two) -> p b two", two=2)

                # one persistent PSUM bank each, halves ping-ponged per group
                nd_all = ndpsum.tile([128, 512], F32, tag="nd", name="ndall")
                proj_all = projpsum.tile([128, 512], F32, tag="pp", name="ppall")

                x1_tiles = [None] * ng
                x0_tiles = [None] * nq

                def emit_x1_dma(G, nsplit=1):
                    x1_tiles[G] = xin.tile([128, KC, 128], F16, tag="xin", name="xint")
                    src = x1g[G * 128 : (G + 1) * 128, :].rearrange(
                        "p (k b) -> p k b", k=KC
                    )
                    step = 128 // nsplit
                    for s0 in range(0, 128, step):
                        nc.sync.dma_start(
                            x1_tiles[G][s0 : s0 + step], src[s0 : s0 + step]
                        )

                def emit_x0_dma(q, nsplit=1):
                    x0_tiles[q] = xg.tile([128, KC, qsz], F16, tag="xg", name="xgt")
                    src = x0g[q * 128 : (q + 1) * 128, :].rearrange(
                        "p (k b) -> p k b", k=KC
                    )
                    step = 128 // nsplit
                    for s0 in range(0, 128, step):
                        nc.sync.dma_start(
                            x0_tiles[q][s0 : s0 + step], src[s0 : s0 + step]
                        )

                def emit_proj_tp(G):
                    pt = proj_all[:, (G % 2) * 256 : (G % 2) * 256 + 256]
                    xt = x1_tiles[G]
                    for k in range(KC):
                        nc.tensor.matmul(
                            pt, lhsT=xt[:, k, :], rhs=wc_sb[:, k, :],
                            start=(k == 0), stop=(k == KC - 1),
                        )
                    nc.vector.tensor_add(tp16[:, G, :], pt, btp_sb)

                def emit_proj_g(q):
                    # borrow an f-pool PSUM slot for the g projection
                    gp = fpsum.tile([128, FTILE * 128], F32, tag="f", name="gpt")
                    gp = gp[:, :qsz]
                    xt = x0_tiles[q]
                    for k in range(KC):
                        nc.tensor.matmul(
                            gp, lhsT=gwt_sb[:, k, :], rhs=xt[:, k, :],
                            start=(k == 0), stop=(k == KC - 1),
                        )
                    nc.vector.tensor_scalar_add(
                        g1v[:, q * qsz : (q + 1) * qsz, 0], gp, gb_sb
                    )

                f_tiles = [None] * n_ftiles
                e_tiles = [None] * n_ftiles
                nd_tiles = [None] * ng
                xr_tiles = [None] * ng

                gpq = max(1, qsz // 128)  # groups per g-projection block

                # tile T = (group G = T//16, slot v = T%16): e-tile col-block
                # j = 4s+p holds row 128G + 64s + 16p + v.
                def tile_rows(T):
                    G, v = T // NSLOT, T % NSLOT
                    return [
                        128 * G + 64 * s + 16 * p + v
                        for s in range(NSTRIP)
                        for p in range(4)
                    ]

                def emit_realign(G, gps_only=False):
                    # theta: ONE natural-order DMA per strip: src = 64
                    # contiguous partitions -> dst [4 parts stride 16,
                    # 16 slots, 128]. phi: 4 DMAs per strip (16 contiguous
                    # src partitions -> one partition, block offset p*128).
                    bi = G % 2
                    phv = phbuf[bi][:, :].rearrange("o (t f) -> o t f", f=4 * D)
                    thv = thbuf[bi][:, :].rearrange("o (t f) -> o t f", f=D)
                    for s in range(NSTRIP):
                        eng = nc.gpsimd if gps_only or s == 0 else nc.sync
                        eng.dma_start(
                            thv[64 * s : 64 * s + 49 : 16, :],
                            tp16[64 * s : 64 * s + 64, G, 0:D],
                        )
                        for p in range(4):
                            eng2 = (
                                nc.gpsimd if gps_only or (s + p) % 2 else nc.sync
                            )
                            o = 64 * s + 16 * p
                            eng2.dma_start(
                                phv[o : o + 1, :, p * D : (p + 1) * D],
                                tp16[o : o + 16, G, D : 2 * D],
                            )

                def emit_outers(T):
                    G, v = T // NSLOT, T % NSLOT
                    f_tiles[T] = fpsum.tile(
                        [128, FTILE * 128], F32, tag="f", name="ftile"
                    )
                    if v == 0:
                        # group-entry hooks: prefetches + nd/xr assignment
                        if 5 <= G + 5 < ng:
                            emit_x1_dma(G + 5)
                        if 3 <= G + 3 < ng:
                            emit_proj_tp(G + 3)
                        # G+1 (not G+2): thbuf[(G+2)%2] is the buffer group G
                        # is still reading -- prefetching 2 ahead would make
                        # this group's outers read the NEXT group's rows.
                        if 1 <= G and G + 1 < ng:
                            emit_realign(G + 1)
                        if (G + 1) % gpq == 0 and (G + 1) // gpq < nq:
                            emit_x0_dma((G + 1) // gpq)
                        if G % gpq == 0 and G > 0:
                            emit_proj_g(G // gpq)
                        nd_tiles[G] = nd_all[:, (G % 2) * 256 : (G % 2) * 256 + 256]
                        xr_tiles[G] = resid.tile([128, C], F16, tag="xr", name="xrt")
                        nc.sync.dma_start(
                            xr_tiles[G], x0r[G * 128 : (G + 1) * 128, :]
                        )
                    # activity filler: a K=128 garbage matmul into the slot,
                    # fully overwritten by the real outers' start=True just
                    # after. Raises warm-clock PE duty so the HAM holds
                    # 2.4 GHz; the 3-buffer f-pool slack absorbs its latency.
                    if FILL_N:
                        nc.tensor.matmul(
                            f_tiles[T][:, 0:FILL_N],
                            lhsT=wbuf[:, 0:D],
                            rhs=wbuf[:, D : D + FILL_N],
                            skip_group_check=True,
                        )
                    bi = G % 2
                    for s in range(NSTRIP):
                        nc.tensor.matmul(
                            f_tiles[T][:, s * 512 : (s + 1) * 512],
                            lhsT=thbuf[bi][64 * s : 64 * s + 64, v * D : (v + 1) * D],
                            rhs=phbuf[bi][
                                64 * s : 64 * s + 64, v * 4 * D : (v + 1) * 4 * D
                            ],
                            tile_position=(64 * s, 0),
                            skip_group_check=True,
                        )

                def emit_exp(T):
                    et = epool.tile([128, FTILE * 128], BF16, tag="e", name="etile")
                    if T % 8 in DVE_TILES:
                        nc.vector.tensor_scalar(
                            et.bitcast(I16)[:, :],
                            f_tiles[T][:, :],
                            SCH_A,
                            SCH_B,
                            mybir.AluOpType.mult,
                            mybir.AluOpType.add,
                        )
                    else:
                        nc.scalar.activation(
                            et[:, :],
                            f_tiles[T][:, :],
                            mybir.ActivationFunctionType.Exp,
                        )
                    e_tiles[T] = et

                def emit_reduces(T):
                    et = e_tiles[T]
                    for j, r in enumerate(tile_rows(T)):
                        G, bl = divmod(r, 128)
                        nc.tensor.matmul(
                            nd_tiles[G][:, 2 * bl : 2 * bl + 2],
                            lhsT=et[:, j * 128 : (j + 1) * 128],
                            rhs=g1[:, 2 * r : 2 * r + 2],
                        )

                def emit_final(G):
                    ot = osb.tile([128, C], F32, tag="ot", name="ott")
                    op = fpsum.tile([128, FTILE * 128], F32, tag="f", name="opt")
                    for h in range(2):
                        nc.tensor.matmul(
                            op[:, h * 512 : (h + 1) * 512],
                            lhsT=y16[:, G * 128 : (G + 1) * 128],
                            rhs=wwt_sb[:, h * 512 : (h + 1) * 512],
                        )
                    for h in range(2):
                        nc.vector.tensor_add(
                            ot[:, h * 512 : (h + 1) * 512],
                            op[:, h * 512 : (h + 1) * 512],
                            xr_tiles[G][:, h * 512 : (h + 1) * 512],
                        )
                    nc.sync.dma_start(out[G * 128 : (G + 1) * 128, :], ot)

                def emit_divide(G):
                    nd = ndsb.tile([128, 256], F32, tag="ndsb")
                    nc.scalar.copy(nd, nd_tiles[G])
                    ndv = nd.rearrange("p (b two) -> p b two", two=2)
                    r = rec.tile([128, 128], F32, tag="rec")
                    # den >= 1 always (the j=i term is exp(f)>=... sum of
                    # positive exps), so the ~51-ULP approx has no edge cases
                    nc.vector.reciprocal_approx_fast(r, ndv[:, :, 1])
                    nc.vector.tensor_mul(
                        y16[:, G * 128 : (G + 1) * 128], ndv[:, :, 0], r
                    )

                def groups_done_at(Tr):
                    # group G's reduces finish at its last slot tile
                    if Tr < 0 or Tr % NSLOT != NSLOT - 1:
                        return []
                    return [Tr // NSLOT]

                # ---- startup: emission order == sync-queue order; the first
                # outer waits x1(G0) -> proj(G0) -> realign(G0). ----
                emit_x1_dma(0, nsplit=4)
                wc_src = wc[:, :].rearrange("p (k d) -> p k d", k=KC)
                for s0 in range(0, 128, 32):
                    nc.sync.dma_start(wc_sb[s0 : s0 + 32], wc_src[s0 : s0 + 32])
                nc.sync.dma_start(btp_sb, btp[:, :])
                # zero-fill realign buffers on the DVE (it is idle until the
                # first projection add; DMA engines stay free for realign)
                nc.vector.memset(phbuf[0], 0.0)
                nc.vector.memset(thbuf[0], 0.0)
                nc.vector.memset(phbuf[1], 0.0)
                nc.vector.memset(thbuf[1], 0.0)
                emit_proj_tp(0)
                # warm-up: garbage matmuls keep the PE active while the
                # realign chain (proj-add -> DMAs) completes
                nwarm = int(os.environ.get("K_WARM", "14"))
                if nwarm:
                    wpsum = fpsum.tile([128, FTILE * 128], F32, tag="f", name="warm")
                    for w in range(nwarm):
                        s = w % NSTRIP
                        nc.tensor.matmul(
                            wpsum[:, s * 512 : (s + 1) * 512],
                            lhsT=wbuf[64 * s : 64 * s + 64, 0:D],
                            rhs=wbuf[64 * s : 64 * s + 64, D : 5 * D],
                            tile_position=(64 * s, 0),
                        )
                emit_realign(0, gps_only=True)
                for Gp in range(1, min(3, ng)):
                    emit_x1_dma(Gp, nsplit=2)
                for Gp in range(3, min(5, ng)):
                    emit_x1_dma(Gp)
                gwt_src = gwt[:, :].rearrange("p (k d) -> p k d", k=KC)
                for s0 in range(0, 128, 64):
                    nc.sync.dma_start(gwt_sb[s0 : s0 + 64], gwt_src[s0 : s0 + 64])
                emit_x0_dma(0, nsplit=2)
                for Gp in range(1, min(3, ng)):
                    emit_proj_tp(Gp)
                if ng > 1:
                    emit_realign(1)
                nc.sync.dma_start(wwt_sb, wwt[:, :])
                nc.sync.dma_start(gb_sb, gb[:, :])

                # software-pipelined emission. exp(T-1) precedes the f-pool
                # allocations of iteration T (finals/outers/gproj) so slot
                # recycling always follows its reader's emission; reduces
                # precede outers so the PE fills any exp-wait with them.
                LAG_RED, LAG_DIV, LAG_FIN = 4, 1, 6
                for T in range(n_ftiles + LAG_RED + LAG_FIN):
                    if 1 <= T <= n_ftiles:
                        emit_exp(T - 1)
                    for G in groups_done_at(T - LAG_RED - LAG_FIN):
                        emit_final(G)
                    if LAG_RED <= T < n_ftiles + LAG_RED:
                        emit_reduces(T - LAG_RED)
                    if T < n_ftiles:
                        emit_outers(T)
                    if T == 1:
                        # deferred init off the critical startup path
                        nc.vector.memset(g1, 1.0)
                        emit_proj_g(0)
                    for G in groups_done_at(T - LAG_RED - LAG_DIV):
                        emit_divide(G)

    nc.compile()
    return nc


_BASS_CACHE = {}


def _get_bass(bc):
    if bc not in _BASS_CACHE:
        _BASS_CACHE[bc] = build_bass(bc)
    return _BASS_CACHE[bc]


def make_core_inputs(x0, x1, g_w, g_b, theta_w, theta_b, phi_w, phi_b, W_w, W_b,
                     bc=None, ncores=NCORES):
    """Host-side preprocessing -> list of per-core input dicts."""
    n = x0.shape[0] if bc is None else bc * ncores
    bc = n // ncores

    x0 = np.asarray(x0, np.float32)[:n]
    x1 = np.asarray(x1, np.float32)[:n]
    x1f = x1.astype(np.float16)
    x0f = x0.astype(np.float16)
    x0r = x0 if not np.any(W_b) else (x0 + np.asarray(W_b, np.float32)[None, :])
    x0r = np.ascontiguousarray(x0r, dtype=np.float16)

    KC = C // 128
    qsz = min(bc, 512)

    # per-partition-contiguous swizzles: arr[G*128+p, k*blk+b] = x[G*blk+b, k*128+p]
    def swizzle(xc, blk):
        g = xc.shape[0] // blk
        a = xc.reshape(g, blk, KC, 128)
        return np.ascontiguousarray(a.transpose(0, 3, 2, 1).reshape(g * 128, KC * blk))

    wc = np.concatenate(
        [np.asarray(theta_w).T, np.asarray(phi_w).T], axis=1
    ).astype(np.float16)  # [C, 2D]
    wcg = np.ascontiguousarray(
        wc.reshape(KC, 128, 2 * D).transpose(1, 0, 2).reshape(128, KC * 2 * D)
    )
    gwt = np.asarray(g_w).T.astype(np.float16)  # [C, D]
    gwtg = np.ascontiguousarray(
        gwt.reshape(KC, 128, D).transpose(1, 0, 2).reshape(128, KC * D)
    )
    import ml_dtypes
    wwt = np.ascontiguousarray(np.asarray(W_w).T.astype(ml_dtypes.bfloat16))  # [D, C]
    btp = np.ascontiguousarray(
        np.tile(np.concatenate([np.asarray(theta_b), np.asarray(phi_b)])[None, :], (128, 1)).astype(np.float32)
    )
    gbc = np.ascontiguousarray(np.asarray(g_b, np.float32).reshape(D, 1))

    in_maps = []
    for c in range(ncores):
        sl = slice(c * bc, (c + 1) * bc)
        in_maps.append(
            {
                "x1g": swizzle(x1f[sl], 128),
                "x0g": swizzle(x0f[sl], qsz),
                "x0r": np.ascontiguousarray(x0r[sl]),
                "wc": wcg,
                "gwt": gwtg,
                "wwt": wwt,
                "btp": btp,
                "gb": gbc,
            }
        )
    return in_maps, bc


def kernel(x0, x1, g_w, g_b, theta_w, theta_b, phi_w, phi_b, W_w, W_b):
    from concourse.bass_utils import run_bass_kernel_spmd

    in_maps, bc = make_core_inputs(
        x0, x1, g_w, g_b, theta_w, theta_b, phi_w, phi_b, W_w, W_b
    )
    nc = _get_bass(bc)
    res = run_bass_kernel_spmd(nc, in_maps, core_ids=list(range(NCORES)))
    outs = [r["out"] for r in res.results]
    return np.ascontiguousarray(np.concatenate(outs, axis=0), dtype=np.float32)
